# revision 11
# baseline (speedup 1.0000x reference)
"""MAGNN model kernel for 8 Trainium2 NeuronCores.

Data-parallel over the batch (512 (user,recipe) pairs per core). Host builds
parameter-only augmented tables; the device does all batch-dependent gathers
(user->recipes->ingredient-blocks, 2-hop) and the attention math.

T_all row [448 f32]: [rec_emb 64 | 1.0 | Pur 4 | Purir 4 | PiBlock 20 | pad->128 | 0.5*ing_emb x5 (320)]

Per core: 4x128 user-row gathers, on-device id extraction + PE transpose to
form per-column offset tiles, then 128 indirect gather columns of T_all
(one column = 4 users x 32 recipes = 128 rows across partitions).
e-logits come from in-row projections; weighted sums are PE matmuls with
block-diagonal alpha as lhsT; softmax denominators ride a ones column.
"""

import numpy as np

NU, NR, NI = 100000, 50000, 8847
D, H, AV = 64, 4, 128
B, RMAX, R20, I5 = 4096, 32, 20, 5
TW = 448


def build_program(upc, ncores):
    import concourse.bass as bass
    import concourse.tile as tile
    from concourse import mybir
    import concourse.bacc as bacc
    import contextlib

    fp32 = mybir.dt.float32
    i32 = mybir.dt.int32
    nchunk = upc // 128
    ncols = upc // 4
    nbank = upc // 32

    nc = bacc.Bacc("TRN2", target_bir_lowering=False, debug=False, num_devices=ncores)

    t_all = nc.dram_tensor("t_all", [NR, TW], fp32, kind="ExternalInput").ap()
    t_uemb = nc.dram_tensor("t_uemb", [NU, D], fp32, kind="ExternalInput").ap()
    t_uids = nc.dram_tensor("t_uids", [NU, RMAX], i32, kind="ExternalInput").ap()
    uf_offs = nc.dram_tensor("uf_offs", [128, nchunk], i32, kind="ExternalInput").ap()
    rf_offs = nc.dram_tensor("rf_offs", [128, nchunk], i32, kind="ExternalInput").ap()
    a_cu = nc.dram_tensor("a_cu", [D, 8], fp32, kind="ExternalInput").ap()
    indsel = nc.dram_tensor("indsel", [128, 4, 32], fp32, kind="ExternalInput").ap()
    indsel2 = nc.dram_tensor("indsel2", [128, 4, 128], fp32, kind="ExternalInput").ap()
    indcol = nc.dram_tensor("indcol", [32, 8, 128], fp32, kind="ExternalInput").ap()
    ind32 = nc.dram_tensor("ind32", [32, 128], fp32, kind="ExternalInput").ap()
    ind32_23 = nc.dram_tensor("ind32_23", [32, 128], fp32, kind="ExternalInput").ap()
    indsum = nc.dram_tensor("indsum", [128, 32], fp32, kind="ExternalInput").ap()
    mask_ur = nc.dram_tensor("mask_ur", [128, 2, 32], fp32, kind="ExternalInput").ap()
    mask_ir = nc.dram_tensor("mask_ir", [128, 2, 32], fp32, kind="ExternalInput").ap()
    w_ut = nc.dram_tensor("w_ut", [64, 4, 128], fp32, kind="ExternalInput").ap()
    wr_efft = nc.dram_tensor("wr_efft", [D, 128], fp32, kind="ExternalInput").ap()
    b_u = nc.dram_tensor("b_u", [128, 1], fp32, kind="ExternalInput").ap()
    q_u = nc.dram_tensor("q_u", [128, 1], fp32, kind="ExternalInput").ap()
    q_r = nc.dram_tensor("q_r", [128, 1], fp32, kind="ExternalInput").ap()
    s0_c = nc.dram_tensor("s0_c", [1, 1], fp32, kind="ExternalInput").ap()
    ident = nc.dram_tensor("ident", [128, 128], fp32, kind="ExternalInput").ap()
    out_d = nc.dram_tensor("out", [upc], fp32, kind="ExternalOutput").ap()

    AF = mybir.ActivationFunctionType
    OP = mybir.AluOpType

    with tile.TileContext(nc) as tc:
        ctx = contextlib.ExitStack()
        with ctx:
            singles = ctx.enter_context(tc.tile_pool(name="singles", bufs=1))
            gpool = ctx.enter_context(tc.tile_pool(name="gath", bufs=3))
            work = ctx.enter_context(tc.tile_pool(name="work", bufs=3))
            ppool = ctx.enter_context(tc.tile_pool(name="ps", bufs=1, space="PSUM"))
            pacc = ctx.enter_context(tc.tile_pool(name="pacc", bufs=1, space="PSUM"))

            _cn = [0]
            def load_const(apx, shape, dtype=fp32):
                _cn[0] += 1
                t = singles.tile(shape, dtype, tag=f"const{_cn[0]}")
                nc.sync.dma_start(out=t[:], in_=apx)
                return t

            sb_acu = load_const(a_cu, [D, 8])
            sb_isel = load_const(indsel, [128, 4, 32])
            sb_isel2 = load_const(indsel2, [128, 4, 128])
            sb_icol = load_const(indcol, [32, 8, 128])
            sb_ind32 = load_const(ind32, [32, 128])
            sb_ind32_23 = load_const(ind32_23, [32, 128])
            sb_indsum = load_const(indsum, [128, 32])
            sb_mur = load_const(mask_ur, [128, 2, 32])
            sb_mir = load_const(mask_ir, [128, 2, 32])
            sb_wut = load_const(w_ut, [64, 4, 128])
            sb_wrt = load_const(wr_efft, [D, 128])
            sb_bu = load_const(b_u, [128, 1])
            sb_qu = load_const(q_u, [128, 1])
            sb_qr = load_const(q_r, [128, 1])
            sb_s0 = load_const(s0_c, [1, 1])
            sb_id = load_const(ident, [128, 128])
            sb_ufo = load_const(uf_offs, [128, nchunk], i32)
            sb_rfo = load_const(rf_offs, [128, nchunk], i32)
            ones_sb = singles.tile([1, 1], fp32)
            nc.vector.memset(ones_sb[:], 1.0)

            # ---- stage 1: user rows + recipe-side rows ----
            u_emb = singles.tile([128, nchunk, D], fp32)
            u_ids = singles.tile([128, nchunk, RMAX], i32)
            r_emb = singles.tile([128, nchunk, 128], fp32)
            for c in range(nchunk):
                nc.gpsimd.indirect_dma_start(
                    out=u_emb[:, c, :], out_offset=None, in_=t_uemb,
                    in_offset=bass.IndirectOffsetOnAxis(ap=sb_ufo[:, c:c + 1], axis=0))
                nc.gpsimd.indirect_dma_start(
                    out=u_ids[:, c, :], out_offset=None, in_=t_uids,
                    in_offset=bass.IndirectOffsetOnAxis(ap=sb_ufo[:, c:c + 1], axis=0))
                nc.gpsimd.indirect_dma_start(
                    out=r_emb[:, c, :], out_offset=None, in_=t_all,
                    in_offset=bass.IndirectOffsetOnAxis(ap=sb_rfo[:, c:c + 1], axis=0))

            # ---- stage 2: transposes + CU + offsets ----
            ids_f = work.tile([128, nchunk, RMAX], fp32, tag="idsf")
            nc.vector.tensor_copy(out=ids_f[:], in_=u_ids[:])
            ids_t = singles.tile([32, nchunk, 128], i32)
            uf_t = singles.tile([D, nchunk, 128], fp32)
            rf_t = singles.tile([D, nchunk, 128], fp32)
            cu_all = singles.tile([128, nchunk, 8], fp32)
            offs = singles.tile([128, ncols], i32)
            for c in range(nchunk):
                pt = ppool.tile([32, 128], fp32, tag="tp_a", space="PSUM")
                nc.tensor.transpose(out=pt[:], in_=ids_f[:, c, :], identity=sb_id[:])
                nc.vector.tensor_copy(out=ids_t[:, c, :], in_=pt[:])
                pu = ppool.tile([D, 128], fp32, tag="tp_b", space="PSUM")
                nc.tensor.transpose(out=pu[:], in_=u_emb[:, c, :], identity=sb_id[:])
                nc.vector.tensor_copy(out=uf_t[:, c, :], in_=pu[:])
                pr = ppool.tile([D, 128], fp32, tag="tp_a", space="PSUM")
                nc.tensor.transpose(out=pr[:], in_=r_emb[:, c, 0:D], identity=sb_id[:])
                nc.vector.tensor_copy(out=rf_t[:, c, :], in_=pr[:])
                pc = ppool.tile([128, 8], fp32, tag="tp_b", space="PSUM")
                nc.tensor.matmul(skip_group_check=True, out=pc[:], lhsT=uf_t[:, c, :], rhs=sb_acu[:], start=True, stop=True)
                nc.vector.tensor_copy(out=cu_all[:, c, :], in_=pc[:])
                idr = ids_t[:, c, :].rearrange("r (jj uu) -> r uu jj", uu=4)
                for uu in range(4):
                    nc.sync.dma_start(out=offs[32 * uu:32 * uu + 32, 32 * c:32 * c + 32],
                                      in_=idr[:, uu, :])

            # ---- stage 3: main stream ----
            wh_all = singles.tile([128, 3, upc], fp32)
            uro_sb = singles.tile([128, nbank, D], fp32)
            iro_sb = singles.tile([128, nbank, D], fp32)
            for g in range(nbank):
                p_ur = pacc.tile([128, 65], fp32, tag="p_ur", space="PSUM")
                p_ir = pacc.tile([128, 65], fp32, tag="p_ir", space="PSUM")
                c = g // 4
                pcb = ppool.tile([32, 8], fp32, tag="misc", space="PSUM")
                nc.tensor.matmul(skip_group_check=True, out=pcb[:], lhsT=sb_isel[:, g % 4, :], rhs=cu_all[:, c, :],
                                 start=True, stop=True)
                cu_b = work.tile([32, 8], fp32, tag="cu_b")
                nc.vector.tensor_copy(out=cu_b[:], in_=pcb[:])
                for jj in range(8):
                    j = 8 * g + jj
                    par = jj % 2
                    po = 32 * (jj // 2)
                    gt = gpool.tile([128, TW], fp32, tag="gt")
                    nc.gpsimd.indirect_dma_start(
                        out=gt[:], out_offset=None, in_=t_all,
                        in_offset=bass.IndirectOffsetOnAxis(ap=offs[:, j:j + 1], axis=0))
                    pcu = ppool.tile([128, 8], fp32, tag="pcu", space="PSUM")
                    nc.tensor.matmul(skip_group_check=True, out=pcu[:], lhsT=sb_icol[:, jj, :], rhs=cu_b[:],
                                     start=True, stop=True)
                    # e_UR = exp(lrelu(cu0 + 0.5*Pur))
                    eur = work.tile([128, 4], fp32, tag="eur")
                    nc.vector.tensor_scalar_mul(out=eur[:], in0=gt[:, 65:69], scalar1=0.5)
                    nc.vector.tensor_add(out=eur[:], in0=eur[:], in1=pcu[:, 0:4])
                    tmp4 = work.tile([128, 4], fp32, tag="tmp4")
                    nc.vector.tensor_scalar_mul(out=tmp4[:], in0=eur[:], scalar1=0.2)
                    nc.vector.tensor_tensor(out=eur[:], in0=eur[:], in1=tmp4[:], op=OP.max)
                    nc.scalar.activation(out=eur[:], in_=eur[:], func=AF.Exp)
                    abd = work.tile([128, 32], fp32, tag="abd")
                    eb = bass.AP(tensor=eur[:].tensor, offset=eur[:].offset,
                                 ap=[eur[:].ap[0], [0, 8], eur[:].ap[-1]])
                    nc.vector.tensor_tensor(out=abd[:], in0=sb_mur[:, par, :], in1=eb, op=OP.mult)
                    nc.tensor.matmul(skip_group_check=True, out=p_ur[po:po + 32, 0:65], lhsT=abd[:], rhs=gt[:, 0:65],
                                     start=(par == 0), stop=(par == 1), tile_position=(0, po))
                    # URIR
                    c3 = work.tile([128, 4], fp32, tag="c3")
                    nc.vector.tensor_scalar_mul(out=c3[:], in0=gt[:, 69:73], scalar1=0.5)
                    nc.vector.tensor_add(out=c3[:], in0=c3[:], in1=pcu[:, 4:8])
                    esum = work.tile([128, 4], fp32, tag="esum")
                    ei_list = []
                    for i in range(I5):
                        ei = work.tile([128, 4], fp32, tag=f"ei{i}")
                        nc.vector.tensor_scalar_mul(out=ei[:], in0=gt[:, 73 + 4 * i:77 + 4 * i],
                                                    scalar1=0.25)
                        nc.vector.tensor_add(out=ei[:], in0=ei[:], in1=c3[:])
                        nc.vector.tensor_scalar_mul(out=tmp4[:], in0=ei[:], scalar1=0.2)
                        nc.vector.tensor_tensor(out=ei[:], in0=ei[:], in1=tmp4[:], op=OP.max)
                        nc.scalar.activation(out=ei[:], in_=ei[:], func=AF.Exp)
                        if i == 0:
                            nc.vector.tensor_copy(out=esum[:], in_=ei[:])
                        else:
                            nc.vector.tensor_add(out=esum[:], in0=esum[:], in1=ei[:])
                        ei_list.append(ei)
                    bbd = work.tile([128, 32], fp32, tag="bbd")
                    esb = bass.AP(tensor=esum[:].tensor, offset=esum[:].offset,
                                  ap=[esum[:].ap[0], [0, 8], esum[:].ap[-1]])
                    nc.vector.tensor_tensor(out=bbd[:], in0=sb_mir[:, par, :], in1=esb, op=OP.mult)
                    nc.tensor.matmul(skip_group_check=True, out=p_ir[po:po + 32, 0:65], lhsT=bbd[:], rhs=gt[:, 0:65],
                                     start=(par == 0), stop=False, tile_position=(0, po))
                    for i in range(I5):
                        aib = work.tile([128, 32], fp32, tag="aib")
                        eib = bass.AP(tensor=ei_list[i][:].tensor, offset=ei_list[i][:].offset,
                                      ap=[ei_list[i][:].ap[0], [0, 8], ei_list[i][:].ap[-1]])
                        nc.vector.tensor_tensor(out=aib[:], in0=sb_mir[:, par, :], in1=eib, op=OP.mult)
                        nc.tensor.matmul(skip_group_check=True, out=p_ir[po:po + 32, 0:64], lhsT=aib[:],
                                         rhs=gt[:, 128 + 64 * i:192 + 64 * i],
                                         start=False, stop=(par == 1 and i == I5 - 1),
                                         tile_position=(0, po))

                # ---- bank epilogue ----
                us = 32 * (g % 4)
                puf = ppool.tile([128, D], fp32, tag="pufx", space="PSUM")
                nc.tensor.matmul(skip_group_check=True, out=puf[:], lhsT=sb_isel2[:, g % 4, :], rhs=u_emb[:, c, :],
                                 start=True, stop=True)
                rec = work.tile([128, 1], fp32, tag="rec")
                t2 = work.tile([128, D], fp32, tag="t2")
                for (acc, dst, sc_uf) in ((p_ur, uro_sb, 0.5), (p_ir, iro_sb, 0.25)):
                    nc.vector.reciprocal(out=rec[:], in_=acc[:, 64:65])
                    nc.vector.tensor_scalar_mul(out=dst[:, g, :], in0=acc[:, 0:64], scalar1=rec[:])
                    nc.vector.tensor_scalar_mul(out=dst[:, g, :], in0=dst[:, g, :], scalar1=0.5)
                    nc.vector.tensor_scalar_mul(out=t2[:], in0=puf[:], scalar1=sc_uf)
                    nc.vector.tensor_add(out=dst[:, g, :], in0=dst[:, g, :], in1=t2[:])
                for k, src3 in enumerate((uro_sb, iro_sb)):
                    pt2 = ppool.tile([D, 128], fp32, tag="tp_a", space="PSUM")
                    nc.tensor.transpose(out=pt2[:], in_=src3[:, g, :], identity=sb_id[:])
                    st = work.tile([D, 128], fp32, tag="st")
                    nc.vector.tensor_copy(out=st[:], in_=pt2[:])
                    pwh = ppool.tile([128, 32], fp32, tag="tp_b", space="PSUM")
                    st_h = st[:, :].rearrange("d (u h) -> d h u", h=4)
                    for h in range(H):
                        nc.tensor.matmul(skip_group_check=True, out=pwh[:], lhsT=sb_wut[:, h, :],
                                         rhs=st_h[:, h, :], start=(h == 0), stop=(h == 3))
                    nc.scalar.activation(out=wh_all[:, k, 32 * g:32 * g + 32], in_=pwh[:],
                                         func=AF.Tanh, bias=sb_bu[:])
                pwr = ppool.tile([128, 32], fp32, tag="tp_b", space="PSUM")
                nc.tensor.matmul(skip_group_check=True, out=pwr[:], lhsT=sb_wrt[:], rhs=rf_t[:, c, us:us + 32],
                                 start=True, stop=True)
                nc.scalar.activation(out=wh_all[:, 2, 32 * g:32 * g + 32], in_=pwr[:], func=AF.Tanh)

            # ---- stage 4: scores ----
            s_sb = singles.tile([1, 3, upc], fp32)
            for k in range(3):
                pss = ppool.tile([1, upc], fp32, tag="tp_a", space="PSUM")
                nc.tensor.matmul(skip_group_check=True, out=pss[:], lhsT=(sb_qr if k == 2 else sb_qu)[:],
                                 rhs=wh_all[:, k, :], start=True, stop=True)
                nc.vector.tensor_copy(out=s_sb[:, k, :], in_=pss[:])
            a0 = work.tile([1, upc], fp32, tag="a0")
            nc.vector.tensor_sub(out=a0[:], in0=s_sb[:, 0, :], in1=s_sb[:, 1, :])
            nc.scalar.activation(out=a0[:], in_=a0[:], func=AF.Sigmoid)
            a1r = work.tile([1, upc], fp32, tag="a1r")
            s0b = bass.AP(tensor=sb_s0[:].tensor, offset=sb_s0[:].offset,
                          ap=[sb_s0[:].ap[0], [0, upc]])
            nc.vector.tensor_sub(out=a1r[:], in0=s_sb[:, 2, :], in1=s0b)
            nc.scalar.activation(out=a1r[:], in_=a1r[:], func=AF.Sigmoid)

            # ---- stage 5: combine + output ----
            out_sb = singles.tile([32, nbank], fp32)
            for g in range(nbank):
                c = g // 4
                us = 32 * (g % 4)
                pa = ppool.tile([32, 2], fp32, tag="misc", space="PSUM")
                nc.tensor.matmul(skip_group_check=True, out=pa[:, 0:1], lhsT=a0[:, 32 * g:32 * g + 32], rhs=ones_sb[:],
                                 start=True, stop=True)
                nc.tensor.matmul(skip_group_check=True, out=pa[:, 1:2], lhsT=a1r[:, 32 * g:32 * g + 32], rhs=ones_sb[:],
                                 start=True, stop=True)
                pa_sb = work.tile([32, 2], fp32, tag="pa_sb")
                nc.vector.tensor_copy(out=pa_sb[:], in_=pa[:])
                pae = ppool.tile([128, 2], fp32, tag="pcu", space="PSUM")
                nc.tensor.matmul(skip_group_check=True, out=pae[:, 0:1], lhsT=sb_ind32[:], rhs=pa_sb[:, 0:1],
                                 start=True, stop=True)
                nc.tensor.matmul(skip_group_check=True, out=pae[:, 1:2], lhsT=sb_ind32_23[:], rhs=pa_sb[:, 1:2],
                                 start=True, stop=True)
                prf = ppool.tile([128, D], fp32, tag="pufx", space="PSUM")
                nc.tensor.matmul(skip_group_check=True, out=prf[:], lhsT=sb_isel2[:, g % 4, :], rhs=r_emb[:, c, 0:D],
                                 start=True, stop=True)
                dif = work.tile([128, D], fp32, tag="dif")
                nc.vector.tensor_sub(out=dif[:], in0=uro_sb[:, g, :], in1=iro_sb[:, g, :])
                nc.vector.tensor_scalar_mul(out=dif[:], in0=dif[:], scalar1=pae[:, 0:1])
                nc.vector.tensor_add(out=dif[:], in0=dif[:], in1=iro_sb[:, g, :])
                hr = work.tile([128, D], fp32, tag="hr")
                nc.vector.tensor_scalar_mul(out=hr[:], in0=prf[:], scalar1=pae[:, 1:2])
                nc.vector.tensor_mul(out=dif[:], in0=dif[:], in1=hr[:])
                rs = work.tile([128, 1], fp32, tag="rs")
                nc.vector.reduce_sum(out=rs[:], in_=dif[:], axis=mybir.AxisListType.X)
                pdot = ppool.tile([32, 1], fp32, tag="misc", space="PSUM")
                nc.tensor.matmul(skip_group_check=True, out=pdot[:], lhsT=sb_indsum[:], rhs=rs[:], start=True, stop=True)
                nc.vector.tensor_copy(out=out_sb[:, g:g + 1], in_=pdot[:])

            nc.sync.dma_start(out=out_d.rearrange("(g u) -> u g", u=32), in_=out_sb[:])

    nc.compile()
    return nc


def host_tables(inputs):
    f = np.float32
    user_emb = np.asarray(inputs["user_emb"], f)
    recipe_emb = np.asarray(inputs["recipe_emb"], f)
    ing_emb = np.asarray(inputs["ingredient_emb"], f)
    u2r = np.asarray(inputs["user2recipes"]).astype(np.int32)
    r2i = np.asarray(inputs["recipe2ingredients"]).astype(np.int32)
    attn_UR = np.asarray(inputs["attn_UR"], f)
    attn_URIR = np.asarray(inputs["attn_URIR"], f)

    Pur = recipe_emb @ attn_UR[:, D:].T
    Purir = recipe_emb @ attn_URIR[:, D:].T
    Pi = ing_emb @ attn_URIR[:, D:].T
    T_all = np.zeros((NR, TW), f)
    T_all[:, :64] = recipe_emb
    T_all[:, 64] = 1.0
    T_all[:, 65:69] = Pur
    T_all[:, 69:73] = Purir
    T_all[:, 73:93] = Pi[r2i].reshape(NR, 20)
    T_all[:, 128:448] = (0.5 * ing_emb[r2i]).reshape(NR, 320)

    A_cu = np.zeros((D, 8), f)
    A_cu[:, 0:4] = (attn_UR[:, :D] + 0.5 * attn_UR[:, D:]).T
    A_cu[:, 4:8] = (attn_URIR[:, :D] + 0.25 * attn_URIR[:, D:]).T

    p = np.arange(128)
    col32 = np.arange(32)
    ind32 = (p[None, :] // 4 == np.arange(32)[:, None]).astype(f)
    indsum = (p[:, None] // 4 == np.arange(32)[None, :]).astype(f)
    # indsel[p_src, v, u'] = (p_src == 32v + u')
    indsel = (p[:, None, None] == 32 * np.arange(4)[None, :, None]
              + np.arange(32)[None, None, :]).astype(f)
    # indsel2[p_src, v, p_dst] = (p_src == 32v + p_dst//4)
    indsel2 = (p[:, None, None] == 32 * np.arange(4)[None, :, None]
               + (p // 4)[None, None, :]).astype(f)
    # indcol[u', jj, p] = (u' == 4jj + p//32)
    indcol = (np.arange(32)[:, None, None] == 4 * np.arange(8)[None, :, None]
              + (p // 32)[None, None, :]).astype(f)
    # mask[p, parity, c(=8u x 4h)] = (c//4 == p//32 + 4*parity)
    mask_ur = (col32[None, None, :] // 4 == p[:, None, None] // 32
               + 4 * np.arange(2)[None, :, None]).astype(f)
    mask_ir = mask_ur * (p[:, None, None] % 32 < R20).astype(f)

    W_u = np.asarray(inputs["W_u"], f)
    w_ut = np.ascontiguousarray(W_u.T.reshape(H, D, AV).transpose(1, 0, 2))
    W_r = np.asarray(inputs["W_r"], f)
    wr_efft = np.ascontiguousarray(((2.0 / 3.0) * W_r.reshape(AV, H, D).sum(1)).T)
    b_r = np.asarray(inputs["b_r"], f)
    q_r = np.asarray(inputs["q_r"], f)
    s0 = np.float32(np.tanh(b_r) @ q_r)

    return dict(
        t_all=T_all, t_uemb=user_emb, t_uids=u2r, a_cu=A_cu,
        indsel=indsel, indsel2=indsel2, indcol=indcol,
        ind32=ind32, ind32_23=((2.0 / 3.0) * ind32).astype(f), indsum=indsum,
        mask_ur=mask_ur, mask_ir=mask_ir, w_ut=w_ut, wr_efft=wr_efft,
        b_u=np.asarray(inputs["b_u"], f).reshape(128, 1),
        q_u=np.asarray(inputs["q_u"], f).reshape(128, 1),
        q_r=q_r.reshape(128, 1).astype(f), s0_c=np.array([[s0]], f),
        ident=np.eye(128, dtype=f),
    )


def make_in_maps(inputs, upc, ncores):
    consts = host_tables(inputs)
    uid = np.asarray(inputs["user_ids"]).astype(np.int32)
    rid = np.asarray(inputs["recipe_ids"]).astype(np.int32)
    nchunk = upc // 128
    in_maps = []
    for k in range(ncores):
        m = dict(consts)
        u = uid[k * upc:(k + 1) * upc]
        r = rid[k * upc:(k + 1) * upc]
        m["uf_offs"] = np.ascontiguousarray(u.reshape(nchunk, 128).T)
        m["rf_offs"] = np.ascontiguousarray(r.reshape(nchunk, 128).T)
        in_maps.append(m)
    return in_maps


_NC_CACHE = {}


def kernel(**inputs):
    from concourse.bass_utils import run_bass_kernel_spmd
    upc, ncores = B // 8, 8
    key = (upc, ncores)
    if key not in _NC_CACHE:
        _NC_CACHE[key] = build_program(upc, ncores)
    nc = _NC_CACHE[key]
    in_maps = make_in_maps(inputs, upc, ncores)
    res = run_bass_kernel_spmd(nc, in_maps, core_ids=list(range(ncores)))
    out = np.concatenate([res.results[k]["out"] for k in range(ncores)])
    return out.astype(np.float32)


# revision 12
# speedup vs baseline: 1.0302x; 1.0302x over previous
"""MAGNN model kernel for 8 Trainium2 NeuronCores.

Data-parallel over the batch (512 (user,recipe) pairs per core). Host builds
parameter-only augmented tables; the device does all batch-dependent gathers
(user->recipes->ingredient-blocks, 2-hop) and the attention math.

T_all row [448 f32]: [rec_emb 64 | 1.0 | Pur 4 | Purir 4 | PiBlock 20 | pad->128 | 0.5*ing_emb x5 (320)]

Per core: 4x128 user-row gathers, on-device id extraction + PE transpose to
form per-column offset tiles, then 128 indirect gather columns of T_all
(one column = 4 users x 32 recipes = 128 rows across partitions).
e-logits come from in-row projections; weighted sums are PE matmuls with
block-diagonal alpha as lhsT; softmax denominators ride a ones column.
"""

import numpy as np

NU, NR, NI = 100000, 50000, 8847
D, H, AV = 64, 4, 128
B, RMAX, R20, I5 = 4096, 32, 20, 5
TW = 448


def build_program(upc, ncores):
    import concourse.bass as bass
    import concourse.tile as tile
    from concourse import mybir
    import concourse.bacc as bacc
    import contextlib

    fp32 = mybir.dt.float32
    i32 = mybir.dt.int32
    nchunk = upc // 128
    ncols = upc // 4
    nbank = upc // 32

    nc = bacc.Bacc("TRN2", target_bir_lowering=False, debug=False, num_devices=ncores)

    t_all = nc.dram_tensor("t_all", [NR, TW], fp32, kind="ExternalInput").ap()
    t_uemb = nc.dram_tensor("t_uemb", [NU, D], fp32, kind="ExternalInput").ap()
    t_uids = nc.dram_tensor("t_uids", [NU, RMAX], i32, kind="ExternalInput").ap()
    uf_offs = nc.dram_tensor("uf_offs", [128, nchunk], i32, kind="ExternalInput").ap()
    rf_offs = nc.dram_tensor("rf_offs", [128, nchunk], i32, kind="ExternalInput").ap()
    a_cu = nc.dram_tensor("a_cu", [D, 8], fp32, kind="ExternalInput").ap()
    indsel = nc.dram_tensor("indsel", [128, 4, 32], fp32, kind="ExternalInput").ap()
    indsel2 = nc.dram_tensor("indsel2", [128, 4, 128], fp32, kind="ExternalInput").ap()
    indcol = nc.dram_tensor("indcol", [32, 8, 128], fp32, kind="ExternalInput").ap()
    ind32 = nc.dram_tensor("ind32", [32, 128], fp32, kind="ExternalInput").ap()
    ind32_23 = nc.dram_tensor("ind32_23", [32, 128], fp32, kind="ExternalInput").ap()
    indsum = nc.dram_tensor("indsum", [128, 32], fp32, kind="ExternalInput").ap()
    mask_ur = nc.dram_tensor("mask_ur", [128, 2, 32], fp32, kind="ExternalInput").ap()
    mask_ir = nc.dram_tensor("mask_ir", [128, 2, 32], fp32, kind="ExternalInput").ap()
    w_ut = nc.dram_tensor("w_ut", [64, 4, 128], fp32, kind="ExternalInput").ap()
    wr_efft = nc.dram_tensor("wr_efft", [D, 128], fp32, kind="ExternalInput").ap()
    b_u = nc.dram_tensor("b_u", [128, 1], fp32, kind="ExternalInput").ap()
    q_u = nc.dram_tensor("q_u", [128, 1], fp32, kind="ExternalInput").ap()
    q_r = nc.dram_tensor("q_r", [128, 1], fp32, kind="ExternalInput").ap()
    s0_c = nc.dram_tensor("s0_c", [1, 1], fp32, kind="ExternalInput").ap()
    ident = nc.dram_tensor("ident", [128, 128], fp32, kind="ExternalInput").ap()
    out_d = nc.dram_tensor("out", [upc], fp32, kind="ExternalOutput").ap()

    AF = mybir.ActivationFunctionType
    OP = mybir.AluOpType

    with tile.TileContext(nc) as tc:
        ctx = contextlib.ExitStack()
        with ctx:
            singles = ctx.enter_context(tc.tile_pool(name="singles", bufs=1))
            gpool = ctx.enter_context(tc.tile_pool(name="gath", bufs=6))
            work = ctx.enter_context(tc.tile_pool(name="work", bufs=4))
            ppool = ctx.enter_context(tc.tile_pool(name="ps", bufs=1, space="PSUM"))
            pacc = ctx.enter_context(tc.tile_pool(name="pacc", bufs=2, space="PSUM"))

            _cn = [0]
            def load_const(apx, shape, dtype=fp32):
                _cn[0] += 1
                t = singles.tile(shape, dtype, tag=f"const{_cn[0]}")
                nc.sync.dma_start(out=t[:], in_=apx)
                return t

            sb_acu = load_const(a_cu, [D, 8])
            sb_isel = load_const(indsel, [128, 4, 32])
            sb_isel2 = load_const(indsel2, [128, 4, 128])
            sb_icol = load_const(indcol, [32, 8, 128])
            sb_ind32 = load_const(ind32, [32, 128])
            sb_ind32_23 = load_const(ind32_23, [32, 128])
            sb_indsum = load_const(indsum, [128, 32])
            sb_mur = load_const(mask_ur, [128, 2, 32])
            sb_mir = load_const(mask_ir, [128, 2, 32])
            sb_wut = load_const(w_ut, [64, 4, 128])
            sb_wrt = load_const(wr_efft, [D, 128])
            sb_bu = load_const(b_u, [128, 1])
            sb_qu = load_const(q_u, [128, 1])
            sb_qr = load_const(q_r, [128, 1])
            sb_s0 = load_const(s0_c, [1, 1])
            sb_id = load_const(ident, [128, 128])
            sb_ufo = load_const(uf_offs, [128, nchunk], i32)
            sb_rfo = load_const(rf_offs, [128, nchunk], i32)
            ones_sb = singles.tile([1, 1], fp32)
            nc.vector.memset(ones_sb[:], 1.0)

            # ---- stage 1: user rows + recipe-side rows ----
            u_emb = singles.tile([128, nchunk, D], fp32)
            u_ids = singles.tile([128, nchunk, RMAX], i32)
            r_emb = singles.tile([128, nchunk, 128], fp32)
            for c in range(nchunk):
                nc.gpsimd.indirect_dma_start(
                    out=u_emb[:, c, :], out_offset=None, in_=t_uemb,
                    in_offset=bass.IndirectOffsetOnAxis(ap=sb_ufo[:, c:c + 1], axis=0))
                nc.gpsimd.indirect_dma_start(
                    out=u_ids[:, c, :], out_offset=None, in_=t_uids,
                    in_offset=bass.IndirectOffsetOnAxis(ap=sb_ufo[:, c:c + 1], axis=0))
                nc.gpsimd.indirect_dma_start(
                    out=r_emb[:, c, :], out_offset=None, in_=t_all,
                    in_offset=bass.IndirectOffsetOnAxis(ap=sb_rfo[:, c:c + 1], axis=0))

            # ---- stage 2: transposes + CU + offsets ----
            ids_f = work.tile([128, nchunk, RMAX], fp32, tag="idsf")
            nc.vector.tensor_copy(out=ids_f[:], in_=u_ids[:])
            ids_t = singles.tile([32, nchunk, 128], i32)
            uf_t = singles.tile([D, nchunk, 128], fp32)
            rf_t = singles.tile([D, nchunk, 128], fp32)
            cu_all = singles.tile([128, nchunk, 8], fp32)
            offs = singles.tile([128, ncols], i32)
            for c in range(nchunk):
                pt = ppool.tile([32, 128], fp32, tag="tp_a", space="PSUM")
                nc.tensor.transpose(out=pt[:], in_=ids_f[:, c, :], identity=sb_id[:])
                nc.vector.tensor_copy(out=ids_t[:, c, :], in_=pt[:])
                pu = ppool.tile([D, 128], fp32, tag="tp_b", space="PSUM")
                nc.tensor.transpose(out=pu[:], in_=u_emb[:, c, :], identity=sb_id[:])
                nc.vector.tensor_copy(out=uf_t[:, c, :], in_=pu[:])
                pr = ppool.tile([D, 128], fp32, tag="tp_a", space="PSUM")
                nc.tensor.transpose(out=pr[:], in_=r_emb[:, c, 0:D], identity=sb_id[:])
                nc.vector.tensor_copy(out=rf_t[:, c, :], in_=pr[:])
                pc = ppool.tile([128, 8], fp32, tag="tp_b", space="PSUM")
                nc.tensor.matmul(skip_group_check=True, out=pc[:], lhsT=uf_t[:, c, :], rhs=sb_acu[:], start=True, stop=True)
                nc.vector.tensor_copy(out=cu_all[:, c, :], in_=pc[:])
                idr = ids_t[:, c, :].rearrange("r (jj uu) -> r uu jj", uu=4)
                for uu in range(4):
                    nc.sync.dma_start(out=offs[32 * uu:32 * uu + 32, 32 * c:32 * c + 32],
                                      in_=idr[:, uu, :])

            # ---- stage 3: main stream ----
            wh_all = singles.tile([128, 3, upc], fp32)
            uro_sb = singles.tile([128, nbank, D], fp32)
            iro_sb = singles.tile([128, nbank, D], fp32)
            for g in range(nbank):
                p_ur = pacc.tile([128, 65], fp32, tag="p_ur", space="PSUM")
                p_ir = pacc.tile([128, 65], fp32, tag="p_ir", space="PSUM")
                c = g // 4
                pcb = ppool.tile([32, 8], fp32, tag="misc", space="PSUM")
                nc.tensor.matmul(skip_group_check=True, out=pcb[:], lhsT=sb_isel[:, g % 4, :], rhs=cu_all[:, c, :],
                                 start=True, stop=True)
                cu_b = work.tile([32, 8], fp32, tag="cu_b")
                nc.vector.tensor_copy(out=cu_b[:], in_=pcb[:])
                for jj in range(8):
                    j = 8 * g + jj
                    par = jj % 2
                    po = 32 * (jj // 2)
                    gt = gpool.tile([128, TW], fp32, tag="gt")
                    nc.gpsimd.indirect_dma_start(
                        out=gt[:], out_offset=None, in_=t_all,
                        in_offset=bass.IndirectOffsetOnAxis(ap=offs[:, j:j + 1], axis=0))
                    pcu = ppool.tile([128, 8], fp32, tag="pcu", space="PSUM")
                    nc.tensor.matmul(skip_group_check=True, out=pcu[:], lhsT=sb_icol[:, jj, :], rhs=cu_b[:],
                                     start=True, stop=True)
                    # e_UR = exp(lrelu(cu0 + 0.5*Pur))
                    eur = work.tile([128, 4], fp32, tag="eur")
                    nc.vector.tensor_scalar_mul(out=eur[:], in0=gt[:, 65:69], scalar1=0.5)
                    nc.vector.tensor_add(out=eur[:], in0=eur[:], in1=pcu[:, 0:4])
                    tmp4 = work.tile([128, 4], fp32, tag="tmp4")
                    nc.vector.tensor_scalar_mul(out=tmp4[:], in0=eur[:], scalar1=0.2)
                    nc.vector.tensor_tensor(out=eur[:], in0=eur[:], in1=tmp4[:], op=OP.max)
                    nc.scalar.activation(out=eur[:], in_=eur[:], func=AF.Exp)
                    abd = work.tile([128, 32], fp32, tag="abd")
                    eb = bass.AP(tensor=eur[:].tensor, offset=eur[:].offset,
                                 ap=[eur[:].ap[0], [0, 8], eur[:].ap[-1]])
                    nc.vector.tensor_tensor(out=abd[:], in0=sb_mur[:, par, :], in1=eb, op=OP.mult)
                    nc.tensor.matmul(skip_group_check=True, out=p_ur[po:po + 32, 0:65], lhsT=abd[:], rhs=gt[:, 0:65],
                                     start=(par == 0), stop=(par == 1), tile_position=(0, po))
                    # URIR
                    c3 = work.tile([128, 4], fp32, tag="c3")
                    nc.vector.tensor_scalar_mul(out=c3[:], in0=gt[:, 69:73], scalar1=0.5)
                    nc.vector.tensor_add(out=c3[:], in0=c3[:], in1=pcu[:, 4:8])
                    esum = work.tile([128, 4], fp32, tag="esum")
                    ei_list = []
                    for i in range(I5):
                        ei = work.tile([128, 4], fp32, tag=f"ei{i}")
                        nc.vector.tensor_scalar_mul(out=ei[:], in0=gt[:, 73 + 4 * i:77 + 4 * i],
                                                    scalar1=0.25)
                        nc.vector.tensor_add(out=ei[:], in0=ei[:], in1=c3[:])
                        nc.vector.tensor_scalar_mul(out=tmp4[:], in0=ei[:], scalar1=0.2)
                        nc.vector.tensor_tensor(out=ei[:], in0=ei[:], in1=tmp4[:], op=OP.max)
                        nc.scalar.activation(out=ei[:], in_=ei[:], func=AF.Exp)
                        if i == 0:
                            nc.vector.tensor_copy(out=esum[:], in_=ei[:])
                        else:
                            nc.vector.tensor_add(out=esum[:], in0=esum[:], in1=ei[:])
                        ei_list.append(ei)
                    bbd = work.tile([128, 32], fp32, tag="bbd")
                    esb = bass.AP(tensor=esum[:].tensor, offset=esum[:].offset,
                                  ap=[esum[:].ap[0], [0, 8], esum[:].ap[-1]])
                    nc.vector.tensor_tensor(out=bbd[:], in0=sb_mir[:, par, :], in1=esb, op=OP.mult)
                    nc.tensor.matmul(skip_group_check=True, out=p_ir[po:po + 32, 0:65], lhsT=bbd[:], rhs=gt[:, 0:65],
                                     start=(par == 0), stop=False, tile_position=(0, po))
                    for i in range(I5):
                        aib = work.tile([128, 32], fp32, tag="aib")
                        eib = bass.AP(tensor=ei_list[i][:].tensor, offset=ei_list[i][:].offset,
                                      ap=[ei_list[i][:].ap[0], [0, 8], ei_list[i][:].ap[-1]])
                        nc.vector.tensor_tensor(out=aib[:], in0=sb_mir[:, par, :], in1=eib, op=OP.mult)
                        nc.tensor.matmul(skip_group_check=True, out=p_ir[po:po + 32, 0:64], lhsT=aib[:],
                                         rhs=gt[:, 128 + 64 * i:192 + 64 * i],
                                         start=False, stop=(par == 1 and i == I5 - 1),
                                         tile_position=(0, po))

                # ---- bank epilogue ----
                us = 32 * (g % 4)
                puf = ppool.tile([128, D], fp32, tag="misc", space="PSUM")
                nc.tensor.matmul(skip_group_check=True, out=puf[:], lhsT=sb_isel2[:, g % 4, :], rhs=u_emb[:, c, :],
                                 start=True, stop=True)
                rec = work.tile([128, 1], fp32, tag="rec")
                t2 = work.tile([128, D], fp32, tag="t2")
                for (acc, dst, sc_uf) in ((p_ur, uro_sb, 0.5), (p_ir, iro_sb, 0.25)):
                    nc.vector.reciprocal(out=rec[:], in_=acc[:, 64:65])
                    nc.vector.tensor_scalar_mul(out=dst[:, g, :], in0=acc[:, 0:64], scalar1=rec[:])
                    nc.vector.tensor_scalar_mul(out=dst[:, g, :], in0=dst[:, g, :], scalar1=0.5)
                    nc.vector.tensor_scalar_mul(out=t2[:], in0=puf[:], scalar1=sc_uf)
                    nc.vector.tensor_add(out=dst[:, g, :], in0=dst[:, g, :], in1=t2[:])
                for k, src3 in enumerate((uro_sb, iro_sb)):
                    pt2 = ppool.tile([D, 128], fp32, tag="tp_a", space="PSUM")
                    nc.tensor.transpose(out=pt2[:], in_=src3[:, g, :], identity=sb_id[:])
                    st = work.tile([D, 128], fp32, tag="st")
                    nc.vector.tensor_copy(out=st[:], in_=pt2[:])
                    pwh = ppool.tile([128, 32], fp32, tag="tp_b", space="PSUM")
                    st_h = st[:, :].rearrange("d (u h) -> d h u", h=4)
                    for h in range(H):
                        nc.tensor.matmul(skip_group_check=True, out=pwh[:], lhsT=sb_wut[:, h, :],
                                         rhs=st_h[:, h, :], start=(h == 0), stop=(h == 3))
                    nc.scalar.activation(out=wh_all[:, k, 32 * g:32 * g + 32], in_=pwh[:],
                                         func=AF.Tanh, bias=sb_bu[:])
                pwr = ppool.tile([128, 32], fp32, tag="tp_b", space="PSUM")
                nc.tensor.matmul(skip_group_check=True, out=pwr[:], lhsT=sb_wrt[:], rhs=rf_t[:, c, us:us + 32],
                                 start=True, stop=True)
                nc.scalar.activation(out=wh_all[:, 2, 32 * g:32 * g + 32], in_=pwr[:], func=AF.Tanh)

            # ---- stage 4: scores ----
            s_sb = singles.tile([1, 3, upc], fp32)
            for k in range(3):
                pss = ppool.tile([1, upc], fp32, tag="tp_a", space="PSUM")
                nc.tensor.matmul(skip_group_check=True, out=pss[:], lhsT=(sb_qr if k == 2 else sb_qu)[:],
                                 rhs=wh_all[:, k, :], start=True, stop=True)
                nc.vector.tensor_copy(out=s_sb[:, k, :], in_=pss[:])
            a0 = work.tile([1, upc], fp32, tag="a0")
            nc.vector.tensor_sub(out=a0[:], in0=s_sb[:, 0, :], in1=s_sb[:, 1, :])
            nc.scalar.activation(out=a0[:], in_=a0[:], func=AF.Sigmoid)
            a1r = work.tile([1, upc], fp32, tag="a1r")
            s0b = bass.AP(tensor=sb_s0[:].tensor, offset=sb_s0[:].offset,
                          ap=[sb_s0[:].ap[0], [0, upc]])
            nc.vector.tensor_sub(out=a1r[:], in0=s_sb[:, 2, :], in1=s0b)
            nc.scalar.activation(out=a1r[:], in_=a1r[:], func=AF.Sigmoid)

            # ---- stage 5: combine + output ----
            out_sb = singles.tile([32, nbank], fp32)
            for g in range(nbank):
                c = g // 4
                us = 32 * (g % 4)
                pa = ppool.tile([32, 2], fp32, tag="misc", space="PSUM")
                nc.tensor.matmul(skip_group_check=True, out=pa[:, 0:1], lhsT=a0[:, 32 * g:32 * g + 32], rhs=ones_sb[:],
                                 start=True, stop=True)
                nc.tensor.matmul(skip_group_check=True, out=pa[:, 1:2], lhsT=a1r[:, 32 * g:32 * g + 32], rhs=ones_sb[:],
                                 start=True, stop=True)
                pa_sb = work.tile([32, 2], fp32, tag="pa_sb")
                nc.vector.tensor_copy(out=pa_sb[:], in_=pa[:])
                pae = ppool.tile([128, 2], fp32, tag="pcu", space="PSUM")
                nc.tensor.matmul(skip_group_check=True, out=pae[:, 0:1], lhsT=sb_ind32[:], rhs=pa_sb[:, 0:1],
                                 start=True, stop=True)
                nc.tensor.matmul(skip_group_check=True, out=pae[:, 1:2], lhsT=sb_ind32_23[:], rhs=pa_sb[:, 1:2],
                                 start=True, stop=True)
                prf = ppool.tile([128, D], fp32, tag="misc", space="PSUM")
                nc.tensor.matmul(skip_group_check=True, out=prf[:], lhsT=sb_isel2[:, g % 4, :], rhs=r_emb[:, c, 0:D],
                                 start=True, stop=True)
                dif = work.tile([128, D], fp32, tag="dif")
                nc.vector.tensor_sub(out=dif[:], in0=uro_sb[:, g, :], in1=iro_sb[:, g, :])
                nc.vector.tensor_scalar_mul(out=dif[:], in0=dif[:], scalar1=pae[:, 0:1])
                nc.vector.tensor_add(out=dif[:], in0=dif[:], in1=iro_sb[:, g, :])
                hr = work.tile([128, D], fp32, tag="hr")
                nc.vector.tensor_scalar_mul(out=hr[:], in0=prf[:], scalar1=pae[:, 1:2])
                nc.vector.tensor_mul(out=dif[:], in0=dif[:], in1=hr[:])
                rs = work.tile([128, 1], fp32, tag="rs")
                nc.vector.reduce_sum(out=rs[:], in_=dif[:], axis=mybir.AxisListType.X)
                pdot = ppool.tile([32, 1], fp32, tag="misc", space="PSUM")
                nc.tensor.matmul(skip_group_check=True, out=pdot[:], lhsT=sb_indsum[:], rhs=rs[:], start=True, stop=True)
                nc.vector.tensor_copy(out=out_sb[:, g:g + 1], in_=pdot[:])

            nc.sync.dma_start(out=out_d.rearrange("(g u) -> u g", u=32), in_=out_sb[:])

    nc.compile()
    return nc


def host_tables(inputs):
    f = np.float32
    user_emb = np.asarray(inputs["user_emb"], f)
    recipe_emb = np.asarray(inputs["recipe_emb"], f)
    ing_emb = np.asarray(inputs["ingredient_emb"], f)
    u2r = np.asarray(inputs["user2recipes"]).astype(np.int32)
    r2i = np.asarray(inputs["recipe2ingredients"]).astype(np.int32)
    attn_UR = np.asarray(inputs["attn_UR"], f)
    attn_URIR = np.asarray(inputs["attn_URIR"], f)

    Pur = recipe_emb @ attn_UR[:, D:].T
    Purir = recipe_emb @ attn_URIR[:, D:].T
    Pi = ing_emb @ attn_URIR[:, D:].T
    T_all = np.zeros((NR, TW), f)
    T_all[:, :64] = recipe_emb
    T_all[:, 64] = 1.0
    T_all[:, 65:69] = Pur
    T_all[:, 69:73] = Purir
    T_all[:, 73:93] = Pi[r2i].reshape(NR, 20)
    T_all[:, 128:448] = (0.5 * ing_emb[r2i]).reshape(NR, 320)

    A_cu = np.zeros((D, 8), f)
    A_cu[:, 0:4] = (attn_UR[:, :D] + 0.5 * attn_UR[:, D:]).T
    A_cu[:, 4:8] = (attn_URIR[:, :D] + 0.25 * attn_URIR[:, D:]).T

    p = np.arange(128)
    col32 = np.arange(32)
    ind32 = (p[None, :] // 4 == np.arange(32)[:, None]).astype(f)
    indsum = (p[:, None] // 4 == np.arange(32)[None, :]).astype(f)
    # indsel[p_src, v, u'] = (p_src == 32v + u')
    indsel = (p[:, None, None] == 32 * np.arange(4)[None, :, None]
              + np.arange(32)[None, None, :]).astype(f)
    # indsel2[p_src, v, p_dst] = (p_src == 32v + p_dst//4)
    indsel2 = (p[:, None, None] == 32 * np.arange(4)[None, :, None]
               + (p // 4)[None, None, :]).astype(f)
    # indcol[u', jj, p] = (u' == 4jj + p//32)
    indcol = (np.arange(32)[:, None, None] == 4 * np.arange(8)[None, :, None]
              + (p // 32)[None, None, :]).astype(f)
    # mask[p, parity, c(=8u x 4h)] = (c//4 == p//32 + 4*parity)
    mask_ur = (col32[None, None, :] // 4 == p[:, None, None] // 32
               + 4 * np.arange(2)[None, :, None]).astype(f)
    mask_ir = mask_ur * (p[:, None, None] % 32 < R20).astype(f)

    W_u = np.asarray(inputs["W_u"], f)
    w_ut = np.ascontiguousarray(W_u.T.reshape(H, D, AV).transpose(1, 0, 2))
    W_r = np.asarray(inputs["W_r"], f)
    wr_efft = np.ascontiguousarray(((2.0 / 3.0) * W_r.reshape(AV, H, D).sum(1)).T)
    b_r = np.asarray(inputs["b_r"], f)
    q_r = np.asarray(inputs["q_r"], f)
    s0 = np.float32(np.tanh(b_r) @ q_r)

    return dict(
        t_all=T_all, t_uemb=user_emb, t_uids=u2r, a_cu=A_cu,
        indsel=indsel, indsel2=indsel2, indcol=indcol,
        ind32=ind32, ind32_23=((2.0 / 3.0) * ind32).astype(f), indsum=indsum,
        mask_ur=mask_ur, mask_ir=mask_ir, w_ut=w_ut, wr_efft=wr_efft,
        b_u=np.asarray(inputs["b_u"], f).reshape(128, 1),
        q_u=np.asarray(inputs["q_u"], f).reshape(128, 1),
        q_r=q_r.reshape(128, 1).astype(f), s0_c=np.array([[s0]], f),
        ident=np.eye(128, dtype=f),
    )


def make_in_maps(inputs, upc, ncores):
    consts = host_tables(inputs)
    uid = np.asarray(inputs["user_ids"]).astype(np.int32)
    rid = np.asarray(inputs["recipe_ids"]).astype(np.int32)
    nchunk = upc // 128
    in_maps = []
    for k in range(ncores):
        m = dict(consts)
        u = uid[k * upc:(k + 1) * upc]
        r = rid[k * upc:(k + 1) * upc]
        m["uf_offs"] = np.ascontiguousarray(u.reshape(nchunk, 128).T)
        m["rf_offs"] = np.ascontiguousarray(r.reshape(nchunk, 128).T)
        in_maps.append(m)
    return in_maps


_NC_CACHE = {}


def kernel(**inputs):
    from concourse.bass_utils import run_bass_kernel_spmd
    upc, ncores = B // 8, 8
    key = (upc, ncores)
    if key not in _NC_CACHE:
        _NC_CACHE[key] = build_program(upc, ncores)
    nc = _NC_CACHE[key]
    in_maps = make_in_maps(inputs, upc, ncores)
    res = run_bass_kernel_spmd(nc, in_maps, core_ids=list(range(ncores)))
    out = np.concatenate([res.results[k]["out"] for k in range(ncores)])
    return out.astype(np.float32)
